# revision 9
# baseline (speedup 1.0000x reference)
"""GQA attention block (B=2,S=2048,H=2048, 16Q/4KV heads, hd=128) on 8 trn2 cores.

Sharding: core i = (batch b = i//4) x (kv-head group g = i%4). Each core
projects its 4 Q heads + 1 KV head from hidden[b], applies RoPE, runs full
softmax attention, and computes a partial o_proj over its 512 attn dims.
Host sums the 4 partials per batch and adds o_b.

All device matmuls are bf16 (fp32 matmul is 4 cyc/row on trn2 PE, bf16 is 1).
Layouts are contraction-major (host passes x.T / w.T). Scores are computed
transposed (key-seq on partitions) so exp'd probs feed the PV matmul without
a transpose; the softmax denominator comes from ones-vector matmuls; 1/den
via ACT ln->exp(-x); the per-column broadcast of 1/den via a K=1 matmul.
"""

import sys

sys.path.insert(0, "/opt/trn_rl_repo")

import math

import ml_dtypes
import numpy as np

import concourse.bass as bass
import concourse.tile as tile
from concourse import bacc, mybir
from concourse.bass_utils import run_bass_kernel_spmd

B, S, H = 2, 2048, 2048
NH, NKV, HD = 16, 4, 128
THETA = 10000.0
NCORES = 8
P = 128
KT = H // P            # 16 contraction tiles over H
NSTRIP = S // 512      # 4 seq strips of 512
NSJ = S // P           # 16 key tiles of 128
QH = NH // NKV         # 4 q heads per core
QD = QH * HD           # 512 q dims per core

F32 = mybir.dt.float32
BF16 = mybir.dt.bfloat16
AF = mybir.ActivationFunctionType
BF = ml_dtypes.bfloat16

LAST_RESULT = None
_NC_CACHE = []


def _cached_program():
    if not _NC_CACHE:
        _NC_CACHE.append(_build_program())
    return _NC_CACHE[0]


def _build_program():
    nc = bacc.Bacc("TRN2", target_bir_lowering=False, debug=False, num_devices=NCORES)

    xT_d = nc.dram_tensor("xT", [H, S], BF16, kind="ExternalInput")
    qwT_d = nc.dram_tensor("qwT", [H, QD], BF16, kind="ExternalInput")
    kwT_d = nc.dram_tensor("kwT", [H, HD], BF16, kind="ExternalInput")
    vwT_d = nc.dram_tensor("vwT", [H, HD], BF16, kind="ExternalInput")
    qb_d = nc.dram_tensor("qb", [P, QH], F32, kind="ExternalInput")
    kb_d = nc.dram_tensor("kb", [P, 1], F32, kind="ExternalInput")
    vb_d = nc.dram_tensor("vb", [1, HD], BF16, kind="ExternalInput")
    owT_d = nc.dram_tensor("owT", [QH, P, H], BF16, kind="ExternalInput")
    cos_d = nc.dram_tensor("cosT", [P, S], F32, kind="ExternalInput")
    sins_d = nc.dram_tensor("sinTs", [P, S], F32, kind="ExternalInput")
    out_d = nc.dram_tensor("outT", [H, S], F32, kind="ExternalOutput")

    inv_sqrt_hd = 1.0 / math.sqrt(HD)

    with tile.TileContext(nc) as tc:
        with (
            tc.tile_pool(name="persist", bufs=1) as persist,
            tc.tile_pool(name="xpool", bufs=2) as xpool,
            tc.tile_pool(name="work", bufs=2) as work,
            tc.tile_pool(name="qrot", bufs=6) as qrotp,
            tc.tile_pool(name="pt", bufs=32) as ptp,
            tc.tile_pool(name="attn", bufs=8) as attnp,
            tc.tile_pool(name="osb", bufs=3) as osbp,
            tc.tile_pool(name="small", bufs=2) as smallp,
            tc.tile_pool(name="ps_scores", bufs=3, space="PSUM") as ps_scores,
            tc.tile_pool(name="ps_pv", bufs=1, space="PSUM") as ps_pv,
            tc.tile_pool(name="ps_den", bufs=1, space="PSUM") as ps_den,
            tc.tile_pool(name="ps_rec", bufs=1, space="PSUM") as ps_rec,
            tc.tile_pool(name="ps_proj", bufs=2, space="PSUM") as ps_proj,
        ):
            # ---- resident weights / tables ----
            qw = persist.tile([P, KT, QD], BF16)
            nc.sync.dma_start(qw[:], qwT_d.rearrange("(k p) n -> p k n", p=P))
            kw = persist.tile([P, KT, HD], BF16)
            nc.sync.dma_start(kw[:], kwT_d.rearrange("(k p) n -> p k n", p=P))
            vw = persist.tile([P, KT, HD], BF16)
            nc.sync.dma_start(vw[:], vwT_d.rearrange("(k p) n -> p k n", p=P))
            ow = persist.tile([P, QH, H], BF16)
            nc.sync.dma_start(ow[:], owT_d.rearrange("h p n -> p h n"))
            cos = persist.tile([P, S], F32)
            nc.sync.dma_start(cos[:], cos_d[:])
            sins = persist.tile([P, S], F32)
            nc.sync.dma_start(sins[:], sins_d[:])
            qb = persist.tile([P, QH], F32)
            nc.sync.dma_start(qb[:], qb_d[:])
            kb = persist.tile([P, 1], F32)
            nc.sync.dma_start(kb[:], kb_d[:])
            vb = persist.tile([1, HD], BF16)
            nc.sync.dma_start(vb[:], vb_d[:])

            ones_col = persist.tile([P, 1], BF16)
            nc.vector.memset(ones_col[:], 1.0)
            ones_row_b = persist.tile([1, P], BF16)
            nc.vector.memset(ones_row_b[:], 1.0)
            ones_row_f = persist.tile([1, P], F32)
            nc.vector.memset(ones_row_f[:], 1.0)

            krot = persist.tile([P, S], BF16)     # rotated K^T (d, sj)
            vnat = persist.tile([P, NSJ, HD], BF16)  # V natural (sj within tile, tile, d)

            def rope(dst_ap, pre, s0):
                # dst = pre*cos + halfswap(pre)*signed_sin (strip cols s0:s0+512).
                # The half swap crosses partitions, which compute engines cannot
                # do (walrus: equal base partitions required) — use DMA.
                t1 = work.tile([P, 512], F32, tag="rope_t1")
                nc.vector.tensor_mul(t1[:], pre[:], cos[:, s0 : s0 + 512])
                sw = work.tile([P, 512], F32, tag="rope_sw")
                nc.gpsimd.dma_start(sw[0:64, :], pre[64:128, :])
                nc.gpsimd.dma_start(sw[64:128, :], pre[0:64, :])
                t2 = work.tile([P, 512], F32, tag="rope_t2")
                nc.vector.tensor_mul(t2[:], sw[:], sins[:, s0 : s0 + 512])
                nc.vector.tensor_add(dst_ap, t1[:], t2[:])

            # ---- phase 1: K and V over all strips ----
            for st in range(NSTRIP):
                s0 = st * 512
                xs = xpool.tile([P, KT, 512], BF16, tag="x")
                nc.sync.dma_start(
                    xs[:], xT_d.rearrange("(k p) s -> p k s", p=P)[:, :, s0 : s0 + 512]
                )
                # K projection -> (d, strip)
                kps = ps_proj.tile([P, 512], F32, tag="proj")
                for k in range(KT):
                    nc.tensor.matmul(
                        kps[:], kw[:, k, :], xs[:, k, :],
                        start=(k == 0), stop=(k == KT - 1),
                    )
                kpre = work.tile([P, 512], F32, tag="kpre")
                nc.scalar.activation(kpre[:], kps[:], AF.Identity, bias=kb[:])
                rope(krot[:, s0 : s0 + 512], kpre, s0)
                # V natural: 4 sj tiles per strip
                for sub in range(4):
                    sj = st * 4 + sub
                    vps = ps_proj.tile([P, HD], F32, tag="proj")
                    nc.tensor.matmul(vps[:], ones_row_b[:], vb[:], start=True, stop=False)
                    for k in range(KT):
                        nc.tensor.matmul(
                            vps[:], xs[:, k, sub * P : (sub + 1) * P], vw[:, k, :],
                            start=False, stop=(k == KT - 1),
                        )
                    nc.vector.tensor_copy(vnat[:, sj, :], vps[:])

            # ---- phase 2: per si-strip: Q proj + RoPE, attention, o_proj ----
            for st in range(NSTRIP):
                s0 = st * 512
                xs = xpool.tile([P, KT, 512], BF16, tag="x")
                nc.sync.dma_start(
                    xs[:], xT_d.rearrange("(k p) s -> p k s", p=P)[:, :, s0 : s0 + 512]
                )
                attn_sb = []
                for h in range(QH):
                    qps = ps_proj.tile([P, 512], F32, tag="proj")
                    for k in range(KT):
                        nc.tensor.matmul(
                            qps[:], qw[:, k, h * P : (h + 1) * P], xs[:, k, :],
                            start=(k == 0), stop=(k == KT - 1),
                        )
                    qpre = work.tile([P, 512], F32, tag="qpre")
                    nc.scalar.activation(qpre[:], qps[:], AF.Identity, bias=qb[:, h : h + 1])
                    qr = qrotp.tile([P, 512], BF16, tag="qrot")
                    rope(qr[:], qpre, s0)

                    # scores^T tiles + exp
                    pts = []
                    for sj in range(NSJ):
                        sps = ps_scores.tile([P, 512], F32, tag="scores")
                        nc.tensor.matmul(
                            sps[:], krot[:, sj * P : (sj + 1) * P], qr[:],
                            start=True, stop=True,
                        )
                        pt = ptp.tile([P, 512], BF16, tag="pt")
                        nc.scalar.activation(pt[:], sps[:], AF.Exp, scale=inv_sqrt_hd)
                        pts.append(pt)
                    # PV and denominator
                    aps = ps_pv.tile([P, 512], F32, tag="pv")
                    for sj in range(NSJ):
                        nc.tensor.matmul(
                            aps[:], vnat[:, sj, :], pts[sj][:],
                            start=(sj == 0), stop=(sj == NSJ - 1),
                        )
                    dps = ps_den.tile([1, 512], F32, tag="den")
                    for sj in range(NSJ):
                        nc.tensor.matmul(
                            dps[:], ones_col[:], pts[sj][:],
                            start=(sj == 0), stop=(sj == NSJ - 1),
                        )
                    dln = smallp.tile([1, 512], F32, tag="dln")
                    nc.scalar.activation(dln[:], dps[:], AF.Ln)
                    rec = smallp.tile([1, 512], F32, tag="rec")
                    nc.scalar.activation(rec[:], dln[:], AF.Exp, scale=-1.0)
                    rps = ps_rec.tile([P, 512], F32, tag="recb")
                    nc.tensor.matmul(rps[:], ones_row_f[:], rec[:], start=True, stop=True)
                    rsb = work.tile([P, 512], F32, tag="rsb")
                    nc.vector.tensor_copy(rsb[:], rps[:])
                    asb = attnp.tile([P, 512], BF16, tag="attn")
                    nc.vector.tensor_mul(asb[:], aps[:], rsb[:])
                    attn_sb.append(asb)

                # partial o_proj for this strip
                for ht in range(KT):
                    ops = ps_proj.tile([P, 512], F32, tag="proj")
                    for h in range(QH):
                        nc.tensor.matmul(
                            ops[:], ow[:, h, ht * P : (ht + 1) * P], attn_sb[h][:],
                            start=(h == 0), stop=(h == QH - 1),
                        )
                    osb = osbp.tile([P, 512], F32, tag="osb")
                    nc.vector.tensor_copy(osb[:], ops[:])
                    nc.sync.dma_start(
                        out_d[ht * P : (ht + 1) * P, s0 : s0 + 512], osb[:]
                    )

    nc.compile()
    return nc


def _rope_tables():
    pos = np.arange(S, dtype=np.float32)
    inv_freq = 1.0 / (THETA ** (np.arange(0, HD, 2, dtype=np.float32) / HD))
    freqs = pos[:, None] * inv_freq[None, :]  # (S, 64)
    cos_h = np.cos(freqs).T.astype(np.float32)  # (64, S)
    sin_h = np.sin(freqs).T.astype(np.float32)
    cosT = np.concatenate([cos_h, cos_h], axis=0)  # (128, S)
    sinTs = np.concatenate([-sin_h, sin_h], axis=0)  # signed
    return cosT, sinTs


def build_in_maps(hidden_states, q_w, q_b, k_w, k_b, v_w, v_b, o_w, o_b):
    hidden_states = np.asarray(hidden_states, dtype=np.float32)
    cosT, sinTs = _rope_tables()

    xT = [np.ascontiguousarray(hidden_states[b].T).astype(BF) for b in range(B)]

    in_maps = []
    for core in range(NCORES):
        b, g = core // NKV, core % NKV
        qs = slice(g * QD, (g + 1) * QD)
        ks = slice(g * HD, (g + 1) * HD)
        qb_t = np.ascontiguousarray(
            q_b[qs].astype(np.float32).reshape(QH, P).T
        )  # (128, 4)
        ow_slice = o_w[:, qs]  # (H, 512)
        owT = np.ascontiguousarray(
            ow_slice.T.reshape(QH, P, H)
        ).astype(BF)  # (4, 128, H)
        in_maps.append(
            {
                "xT": xT[b],
                "qwT": np.ascontiguousarray(q_w[qs].T).astype(BF),
                "kwT": np.ascontiguousarray(k_w[ks].T).astype(BF),
                "vwT": np.ascontiguousarray(v_w[ks].T).astype(BF),
                "qb": qb_t,
                "kb": np.asarray(k_b[ks], dtype=np.float32).reshape(P, 1),
                "vb": np.asarray(v_b[ks]).astype(BF).reshape(1, HD),
                "owT": owT,
                "cosT": cosT,
                "sinTs": sinTs,
            }
        )
    return in_maps


def kernel(hidden_states, q_w, q_b, k_w, k_b, v_w, v_b, o_w, o_b):
    global LAST_RESULT
    in_maps = build_in_maps(
        hidden_states, q_w, q_b, k_w, k_b, v_w, v_b, o_w, o_b
    )
    nc = _cached_program()
    res = run_bass_kernel_spmd(nc, in_maps, list(range(NCORES)))
    LAST_RESULT = res
    o_b = np.asarray(o_b, dtype=np.float32)

    out = np.empty((B, S, H), dtype=np.float32)
    ob = o_b
    for b in range(B):
        acc = np.zeros((H, S), dtype=np.float32)
        for g in range(NKV):
            acc += res.results[b * NKV + g]["outT"]
        out[b] = acc.T + ob[None, :]
    return out
